# revision 5
# baseline (speedup 1.0000x reference)
"""DiagSSMBlock Trainium2 kernel.

h_t = sum_{k=0..t} a^k * (B^T x_{t-k})  ==  h_t = a * h_{t-1} + s_t, s = B^T x^T.

Strategy: shard T across the 8 cores (1024 steps each + 32-step halo; |a| <=
sqrt(2/1024) ~ 0.044 so a^32 < 1e-43 == 0 in fp32, making slabs exactly
independent).  Host passes x pre-transposed ([H, T_slab]) so the tensor engine
can contract over H with no on-chip transposes; the scan output is returned
channel-major [H, T_slab] bf16 and upcast/transposed back on host.

Matmul operands are bf16 (tolerance 2e-2; bf16 adds ~3e-3): halves input DMA
vs fp32 and enables Fast Weight Load, so LDWEIGHTS hides completely under the
matmuls instead of being the PE bottleneck (fp32r weight loads were 176ns vs
147ns matmuls).  PSUM accumulation and the scan state stay fp32.

DMA issue cost dominates scheduling: each dma_start costs ~600ns of HWDGE
descriptor-gen serialized on the issuing sequencer (measured), so inputs are
batched into a few multi-MB 3D-AP transfers (b host-rearranged to [g, p, kq,
c] making each group slab contiguous 2KB lines; x in 4 slabs) and issue-order
matches PE consumption order.  sync carries x + a, scalar carries b then the
output stores.  Time is chunked (480, 480, 96) per group: psum tiles stay
under the 512-fp32 bank limit and the final chunk's scan+store tail is short.
"""

import sys

if "/opt/trn_rl_repo" not in sys.path:
    sys.path.insert(0, "/opt/trn_rl_repo")

import numpy as np

T, H = 8192, 1024
NC = 8
P = 128
T_LOC = T // NC            # 1024 output timesteps per core
HALO = 32                  # scan warmup; a^32 == 0 in fp32
W = T_LOC + HALO           # 1056
CHUNKS = ((0, 480), (480, 480), (960, 96))
KQ = H // P                # 8 contraction chunks
G = H // P                 # 8 channel groups
N_WARM = 10                # dummy matmuls to lift the HAM clock gate

_state = {}


def _build_nc():
    import concourse.tile as tile
    from concourse import bacc, mybir

    bf16 = mybir.dt.bfloat16
    f32 = mybir.dt.float32

    nc = bacc.Bacc("TRN2", target_bir_lowering=False, debug=False, num_devices=NC)
    xt_e = nc.dram_tensor("xt", [H, W], bf16, kind="ExternalInput").ap()
    # host layout: row g*128+p, col kq*128+c  (== b[kq*128+p, g*128+c])
    b_e = nc.dram_tensor("b", [H, H], bf16, kind="ExternalInput").ap()
    av_e = nc.dram_tensor("av", [P, G], f32, kind="ExternalInput").ap()
    out_e = nc.dram_tensor("out", [H, T_LOC], bf16, kind="ExternalOutput").ap()
    flush_e = nc.dram_tensor("warm_flush", [P, 1], f32).ap()

    with tile.TileContext(nc) as tc:
        with (
            tc.tile_pool(name="consts", bufs=1) as consts,
            tc.tile_pool(name="bpool", bufs=1) as bpool,
            tc.tile_pool(name="xpool", bufs=1) as xpool,
            tc.tile_pool(name="hpool", bufs=1) as hpool,
            tc.tile_pool(name="psb", bufs=5, space="PSUM") as psb,
            tc.tile_pool(name="pss", bufs=2, space="PSUM") as pss,
            tc.tile_pool(name="warmps", bufs=1, space="PSUM") as warmps,
        ):
            # PE warm-up: dummy bf16 matmuls gated only on a gpsimd memset so
            # the HAM clock-gate's 3.4us warm window starts during the
            # input-DMA ramp.
            warm_sb = consts.tile([P, P], bf16, tag="warm")
            nc.gpsimd.memset(warm_sb[:], 0.0)
            wps = warmps.tile([P, P], f32)
            for i in range(N_WARM):
                nc.tensor.matmul(
                    wps[:],
                    warm_sb[:],
                    warm_sb[:],
                    start=(i == 0),
                    stop=(i == N_WARM - 1),
                )
            flush_sb = consts.tile([P, 1], f32, tag="flush")
            nc.vector.tensor_copy(flush_sb[:], wps[:, 0:1])
            nc.gpsimd.dma_start(flush_e[:], flush_sb[:])

            # x slabs on sync, issue-ordered by PE consumption; kq is a free
            # dim so each slab is one descriptor-gen instruction.
            def xt_load(kq0, nkq, n0, ch, tag):
                t = xpool.tile([P, nkq, ch], bf16, tag=tag)
                nc.sync.dma_start(
                    t[:],
                    xt_e[kq0 * P : (kq0 + nkq) * P, n0 : n0 + ch].rearrange(
                        "(k p) c -> p k c", k=nkq
                    ),
                )
                return t

            xt_a = xt_load(0, 4, 0, 480, "xa")        # chunk 0, kq 0-3
            xt_b = xt_load(4, 4, 0, 480, "xb")        # chunk 0, kq 4-7
            xt_c = xt_load(0, 8, 480, 480, "xc")      # chunk 1
            av_sb = consts.tile([P, G], f32, tag="av")
            nc.sync.dma_start(av_sb[:], av_e[:])
            xt_d = xt_load(0, 8, 960, 96, "xd")       # chunk 2

            def x_slice(kq, ni, ch):
                if ni == 0:
                    return (xt_a if kq < 4 else xt_b)[:, kq % 4, :]
                t = xt_c if ni == 1 else xt_d
                return t[:, kq, :]

            # b group slabs on scalar: g0 alone (fast start), then 2/2/3.
            b_sl = [None] * G
            for g0, ng, tag in ((0, 1, "b0"), (1, 2, "b12"), (3, 2, "b34"), (5, 3, "b57")):
                bt = bpool.tile([P, ng, KQ * P], bf16, tag=tag)
                nc.scalar.dma_start(
                    bt[:],
                    b_e[g0 * P : (g0 + ng) * P, :].rearrange(
                        "(g p) c -> p g c", g=ng
                    ),
                )
                for g in range(g0, g0 + ng):
                    b_sl[g] = (bt, g - g0)

            def b_slice(kq, g):
                bt, j = b_sl[g]
                return bt[:, j, kq * P : (kq + 1) * P]

            for g in range(G):
                h_t = hpool.tile([P, W], bf16, tag=f"h{g}")
                for ni, (n0, ch) in enumerate(CHUNKS):
                    ps = (psb if ch == 480 else pss).tile([P, ch], f32)
                    for kq in range(KQ):
                        nc.tensor.matmul(
                            ps[:],
                            b_slice(kq, g),
                            x_slice(kq, ni, ch),
                            start=(kq == 0),
                            stop=(kq == KQ - 1),
                        )
                    init = 0.0 if ni == 0 else h_t[:, n0 - 1 : n0]
                    nc.vector.tensor_tensor_scan(
                        h_t[:, n0 : n0 + ch],
                        av_sb[:, g : g + 1].to_broadcast((P, ch)),
                        ps[:],
                        init,
                        op0=mybir.AluOpType.mult,
                        op1=mybir.AluOpType.add,
                    )
                    if ni == 1:
                        nc.scalar.dma_start(
                            out_e[g * P : (g + 1) * P, 0 : 960 - HALO],
                            h_t[:, HALO:960],
                        )
                    elif ni == 2:
                        nc.scalar.dma_start(
                            out_e[g * P : (g + 1) * P, 960 - HALO : T_LOC],
                            h_t[:, 960:W],
                        )

    nc.compile()
    return nc


def _get_nc():
    if "nc" not in _state:
        _state["nc"] = _build_nc()
    return _state["nc"]


def _shard_inputs(x_seq, a_diag, b_mat):
    import ml_dtypes

    bf16 = ml_dtypes.bfloat16
    x = np.asarray(x_seq, dtype=np.float32)
    a = np.asarray(a_diag, dtype=np.float32)
    b = np.asarray(b_mat, dtype=np.float32)
    x_pad = np.concatenate([np.zeros((HALO, H), np.float32), x], axis=0)
    xT = np.ascontiguousarray(x_pad.T.astype(bf16))  # [H, T + HALO]
    # [kq, p, g, c] -> [g, p, kq, c]: row g*128+p, col kq*128+c
    b_resh = np.ascontiguousarray(
        b.reshape(KQ, P, G, P).transpose(2, 1, 0, 3).reshape(H, H).astype(bf16)
    )
    av = np.ascontiguousarray(a.reshape(G, P).T)  # [P, G]
    in_maps = []
    for i in range(NC):
        in_maps.append(
            {
                "xt": np.ascontiguousarray(xT[:, i * T_LOC : i * T_LOC + W]),
                "b": b_resh,
                "av": av,
            }
        )
    return in_maps


def kernel(x_seq, a_diag, b_mat):
    from concourse.bass_utils import run_bass_kernel_spmd

    nc = _get_nc()
    in_maps = _shard_inputs(x_seq, a_diag, b_mat)
    res = run_bass_kernel_spmd(nc, in_maps, list(range(NC)))
    _state["last_result"] = res
    out = np.concatenate(
        [
            np.asarray(res.results[i]["out"]).astype(np.float32).T
            for i in range(NC)
        ],
        axis=0,
    )
    return out


# revision 11
# speedup vs baseline: 1.1080x; 1.1080x over previous
"""DiagSSMBlock Trainium2 kernel.

h_t = sum_{k=0..t} a^k * (B^T x_{t-k})  ==  h_t = a * h_{t-1} + s_t, s = B^T x^T.

Strategy: shard T across the 8 cores (1024 steps each + 32-step halo; |a| <=
sqrt(2/1024) ~ 0.044 so a^32 < 1e-43 == 0 in fp32, making slabs exactly
independent).  Host passes x pre-transposed ([H, T_slab]) so the tensor engine
can contract over H with no on-chip transposes; the scan output is returned
channel-major [H, T_slab] bf16 and upcast/transposed back on host.

Matmul operands are bf16 (tolerance 2e-2; bf16 adds ~3e-3): halves input DMA
vs fp32 and enables Fast Weight Load, so LDWEIGHTS hides completely under the
matmuls instead of being the PE bottleneck (fp32r weight loads were 176ns vs
147ns matmuls).  PSUM accumulation and the scan state stay fp32.

DMA issue cost dominates scheduling: each dma_start costs ~600ns of HWDGE
descriptor-gen serialized on the issuing sequencer (measured), so inputs are
batched into a few multi-MB 3D-AP transfers (b host-rearranged to [g, p, kq,
c] making each group slab contiguous 2KB lines; x in 4 slabs) and issue-order
matches PE consumption order.  sync carries x + a, scalar carries b then the
output stores.  Time is chunked (480, 480, 96) per group: psum tiles stay
under the 512-fp32 bank limit and the final chunk's scan+store tail is short.
"""

import sys

if "/opt/trn_rl_repo" not in sys.path:
    sys.path.insert(0, "/opt/trn_rl_repo")

import numpy as np

T, H = 8192, 1024
NC = 8
P = 128
T_LOC = T // NC            # 1024 output timesteps per core
HALO = 32                  # scan warmup; a^32 == 0 in fp32
W = T_LOC + HALO           # 1056
CHUNKS = ((0, 480), (480, 480), (960, 96))
KQ = H // P                # 8 contraction chunks
G = H // P                 # 8 channel groups
N_WARM = 10                # dummy matmuls to lift the HAM clock gate

_state = {}


def _build_nc():
    import concourse.tile as tile
    from concourse import bacc, mybir

    bf16 = mybir.dt.bfloat16
    f32 = mybir.dt.float32

    nc = bacc.Bacc("TRN2", target_bir_lowering=False, debug=False, num_devices=NC)
    # x slab with the a-vector appended as 8 extra columns (avoids a separate
    # small strided DMA whose descriptor-gen measured 3.65us)
    xt_e = nc.dram_tensor("xt", [H, W + G], bf16, kind="ExternalInput").ap()
    # host layout: row g*128+p, col kq*128+c  (== b[kq*128+p, g*128+c])
    b_e = nc.dram_tensor("b", [H, H], bf16, kind="ExternalInput").ap()
    out_e = nc.dram_tensor("out", [H, T_LOC], bf16, kind="ExternalOutput").ap()

    with tile.TileContext(nc) as tc:
        with (
            tc.tile_pool(name="consts", bufs=1) as consts,
            tc.tile_pool(name="bpool", bufs=1) as bpool,
            tc.tile_pool(name="xpool", bufs=1) as xpool,
            tc.tile_pool(name="hpool", bufs=1) as hpool,
            tc.tile_pool(name="psb", bufs=5, space="PSUM") as psb,
            tc.tile_pool(name="pss", bufs=2, space="PSUM") as pss,
            tc.tile_pool(name="warmps", bufs=1, space="PSUM") as warmps,
        ):
            # PE warm-up: dummy bf16 matmuls gated only on a DVE memset so the
            # HAM clock-gate's 3.4us warm window starts during the input-DMA
            # ramp.  No gpsimd anywhere: a gpsimd (SWDGE) flush DMA's
            # end-block DRAIN measured 12us and collapsed HWDGE DMA
            # throughput while it polled.
            warm_sb = consts.tile([P, P], bf16, tag="warm")
            nc.vector.memset(warm_sb[:], 0.0)
            wps = warmps.tile([P, P], f32)
            for i in range(N_WARM):
                nc.tensor.matmul(
                    wps[:],
                    warm_sb[:],
                    warm_sb[:],
                    start=(i == 0),
                    stop=(i == N_WARM - 1),
                )
            flush_sb = consts.tile([P, 1], f32, tag="flush")
            nc.vector.tensor_copy(flush_sb[:], wps[:, 0:1])

            # x slabs on sync, issue-ordered by PE consumption; kq is a free
            # dim so each slab is one descriptor-gen instruction.
            def xt_load(kq0, nkq, n0, ch, tag):
                t = xpool.tile([P, nkq, ch], bf16, tag=tag)
                nc.sync.dma_start(
                    t[:],
                    xt_e[kq0 * P : (kq0 + nkq) * P, n0 : n0 + ch].rearrange(
                        "(k p) c -> p k c", k=nkq
                    ),
                )
                return t

            xt_a = xt_load(0, 4, 0, 480, "xa")        # chunk 0, kq 0-3
            xt_b = xt_load(4, 4, 0, 480, "xb")        # chunk 0, kq 4-7
            xt_c = xt_load(0, 8, 480, 480, "xc")      # chunk 1
            xt_d = xt_load(0, 8, 960, 96 + G, "xd")   # chunk 2 + a columns
            av_ap = xt_d[:, 0, 96 : 96 + G]           # [P, G] bf16

            def x_slice(kq, ni, ch):
                if ni == 0:
                    return (xt_a if kq < 4 else xt_b)[:, kq % 4, :]
                if ni == 1:
                    return xt_c[:, kq, :]
                return xt_d[:, kq, 0:96]

            # b group slabs on scalar: g0 alone (fast start), then 2/2/3.
            b_sl = [None] * G
            for g0, ng, tag in ((0, 1, "b0"), (1, 2, "b12"), (3, 2, "b34"), (5, 3, "b57")):
                bt = bpool.tile([P, ng, KQ * P], bf16, tag=tag)
                nc.scalar.dma_start(
                    bt[:],
                    b_e[g0 * P : (g0 + ng) * P, :].rearrange(
                        "(g p) c -> p g c", g=ng
                    ),
                )
                for g in range(g0, g0 + ng):
                    b_sl[g] = (bt, g - g0)

            def b_slice(kq, g):
                bt, j = b_sl[g]
                return bt[:, j, kq * P : (kq + 1) * P]

            for g in range(G):
                h_t = hpool.tile([P, W], bf16, tag=f"h{g}")
                for ni, (n0, ch) in enumerate(CHUNKS):
                    ps = (psb if ch == 480 else pss).tile([P, ch], f32)
                    for kq in range(KQ):
                        nc.tensor.matmul(
                            ps[:],
                            b_slice(kq, g),
                            x_slice(kq, ni, ch),
                            start=(kq == 0),
                            stop=(kq == KQ - 1),
                        )
                    init = 0.0 if ni == 0 else h_t[:, n0 - 1 : n0]
                    nc.vector.tensor_tensor_scan(
                        h_t[:, n0 : n0 + ch],
                        av_ap[:, g : g + 1].to_broadcast((P, ch)),
                        ps[:],
                        init,
                        op0=mybir.AluOpType.mult,
                        op1=mybir.AluOpType.add,
                    )
                    if ni == 1:
                        nc.sync.dma_start(
                            out_e[g * P : (g + 1) * P, 0 : 960 - HALO],
                            h_t[:, HALO:960],
                        )
                    elif ni == 2:
                        nc.sync.dma_start(
                            out_e[g * P : (g + 1) * P, 960 - HALO : T_LOC],
                            h_t[:, 960:W],
                        )

    nc.compile()
    return nc


def _get_nc():
    if "nc" not in _state:
        _state["nc"] = _build_nc()
    return _state["nc"]


def _shard_inputs(x_seq, a_diag, b_mat):
    import ml_dtypes

    bf16 = ml_dtypes.bfloat16
    x = np.asarray(x_seq, dtype=np.float32)
    a = np.asarray(a_diag, dtype=np.float32)
    b = np.asarray(b_mat, dtype=np.float32)
    x_pad = np.concatenate([np.zeros((HALO, H), np.float32), x], axis=0)
    xT = x_pad.T.astype(bf16)  # [H, T + HALO]
    # [kq, p, g, c] -> [g, p, kq, c]: row g*128+p, col kq*128+c
    b_resh = np.ascontiguousarray(
        b.reshape(KQ, P, G, P).transpose(2, 1, 0, 3).reshape(H, H).astype(bf16)
    )
    # a-columns appended to each x slab: row r, col j -> a[j*128 + r%128]
    av_cols = np.tile(a.reshape(G, P).T, (KQ, 1)).astype(bf16)  # [H, G]
    in_maps = []
    for i in range(NC):
        in_maps.append(
            {
                "xt": np.ascontiguousarray(
                    np.concatenate(
                        [xT[:, i * T_LOC : i * T_LOC + W], av_cols], axis=1
                    )
                ),
                "b": b_resh,
            }
        )
    return in_maps


def kernel(x_seq, a_diag, b_mat):
    from concourse.bass_utils import run_bass_kernel_spmd

    nc = _get_nc()
    in_maps = _shard_inputs(x_seq, a_diag, b_mat)
    res = run_bass_kernel_spmd(nc, in_maps, list(range(NC)))
    _state["last_result"] = res
    out = np.concatenate(
        [
            np.asarray(res.results[i]["out"]).astype(np.float32).T
            for i in range(NC)
        ],
        axis=0,
    )
    return out
